# revision 2
# baseline (speedup 1.0000x reference)
import sys
if "/opt/trn_rl_repo" not in sys.path:
    sys.path.insert(0, "/opt/trn_rl_repo")
"""Routed (top-2 gather) MoE Bass/Tile kernel builder.

Per-core program: x shard [NTOK, H] -> 2 MoE layers -> y [NTOK+1, H]
(row NTOK is a trash row for pad scatter-adds; host slices it off).

ln_g/ln_b are folded into Wr/br/We/be on the host (fp64), so the device
computes zraw = (x-mu)*rsig only.

Layer dataflow:
  Phase A (per 128-token tile, software-pipelined):
    - DMA x tile [128, H]; write it straight to dst rows (residual init)
    - LN stats -> mu, rsig; z32 = (x-mu)*rsig f32; zb = same in bf16
    - PE-transpose z32 -> zf f32 (router input)
    - router logits via fp32 PE matmuls (exact); layer-2 via host surrogate
    - top-2 renormalized weights w [128, E]
    - incremental compaction: within-tile exclusive prefix via strict-upper-
      triangular PE matmul + running per-expert base offsets (PSUM-accumulated
      broadcast); all integer-exact in f32
    - per slot s in {0,1}: dest/w/tokidx via mask-select; SWDGE scatter of
      zb rows -> zc_dram[dest] and (w, tokidx) -> wc_dram[dest]
  Phase B (per expert e, capacity C tokens = NCT tiles):
    - linear load zcb [128, H] bf16 + wcr [128, 2]; PE-transpose -> zcT bf16
    - per ho: stream We' bf16 in half-K chunks; psum += zcT_k @ We_k over k
    - drain: ACT scales psum by w (per-partition); DVE adds w*be row;
      SWDGE scatter-ADD [128, HO] into dst rows (pre-initialized with x)
"""

import numpy as np

import concourse.bass as bass
import concourse.bacc as bacc
import concourse.mybir as mybir
import concourse.tile as tile
from concourse import masks

F32 = mybir.dt.float32
BF16 = mybir.dt.bfloat16
I32 = mybir.dt.int32
AF = mybir.ActivationFunctionType
ALU = mybir.AluOpType
AX = mybir.AxisListType

LN_EPS = 1e-5


def build_moe_routed_kernel(NTOK, H, E, L, HO=512, C=640):
    """Inputs: x [NTOK,H] f32, Wr [L,H,E] f32 (g-folded), br [L,E] f32,
    We [L,E,H,H] bf16 (g-folded), be [L,E,H] f32, Ucomb [H,NU] f32,
    rconst [8,E] f32. Output: y [NTOK+1, H] f32 (last row trash)."""
    assert NTOK % 128 == 0 and H % 128 == 0 and H % HO == 0 and C % 128 == 0
    KT = H // 128
    NI = NTOK // 128
    NHO = H // HO
    NCT = C // 128
    EC = E * C
    ZROWS = EC + 4 * 128          # slack rows against capacity overflow
    TB = 4
    KH = KT // 2                  # half-K weight chunk
    assert KT % TB == 0 and KH % TB == 0
    NU = 4 + 4 * E + E
    assert L == 2

    nc = bacc.Bacc("TRN2", target_bir_lowering=False, debug=False)
    x_d = nc.declare_dram_parameter("x", [NTOK, H], F32, False)
    wr_d = nc.declare_dram_parameter("Wr", [L, H, E], F32, False)
    br_d = nc.declare_dram_parameter("br", [L, E], F32, False)
    we_d = nc.declare_dram_parameter("We", [L, E, H, H], BF16, False)
    be_d = nc.declare_dram_parameter("be", [L, E, H], F32, False)
    uc_d = nc.declare_dram_parameter("Ucomb", [H, NU], F32, False)
    rc_d = nc.declare_dram_parameter("rconst", [8, E], F32, False)
    y_d = nc.declare_dram_parameter("y", [NTOK + 1, H], F32, True)
    x1_d = nc.dram_tensor("x1_scratch", [NTOK + 1, H], F32)
    zc_d = [nc.dram_tensor(f"zc{l}", [ZROWS, H], BF16) for l in range(L)]
    wc_d = [nc.dram_tensor(f"wc{l}", [ZROWS, 2], F32) for l in range(L)]
    assert ZROWS % 128 == 0

    with tile.TileContext(nc) as tc:
        with (
            tc.tile_pool(name="const", bufs=1) as constp,
            tc.tile_pool(name="lcon", bufs=2) as lconp,
            tc.tile_pool(name="xin", bufs=2) as xp,
            tc.tile_pool(name="z32", bufs=2) as z32p,
            tc.tile_pool(name="zb", bufs=3) as zbp,
            tc.tile_pool(name="zf", bufs=1) as zfp,
            tc.tile_pool(name="small", bufs=4 * NI) as smp,
            tc.tile_pool(name="wrout", bufs=3 * NI) as wp,
            tc.tile_pool(name="cmp", bufs=4) as cmpp,
            tc.tile_pool(name="zcT", bufs=NCT + 1) as ztp,
            tc.tile_pool(name="zcb", bufs=2) as zcbp,
            tc.tile_pool(name="wch", bufs=3) as wchp,
            tc.tile_pool(name="tm", bufs=3) as tmp_p,
            tc.tile_pool(name="ps", bufs=8, space="PSUM") as psp,
        ):
            ident = constp.tile([128, 128], F32)
            masks.make_identity(nc, ident[:])
            identb = constp.tile([128, 128], BF16)
            nc.vector.tensor_copy(identb[:], ident[:])
            eps_t = constp.tile([128, 1], F32)
            nc.gpsimd.memset(eps_t[:], LN_EPS)
            # strictly-upper-triangular ones: ut[c, p] = 1 iff c < p
            ut128 = constp.tile([128, 128], F32)
            masks.make_upper_triangular(nc, ut128[:], val=1.0, diag=False)
            ones_col = constp.tile([128, 1], F32)
            nc.gpsimd.memset(ones_col[:], 1.0)
            ones_row = constp.tile([1, 128], F32)
            nc.gpsimd.memset(ones_row[:], 1.0)
            # token index within tile
            tok_i32 = constp.tile([128, 1], I32)
            nc.gpsimd.iota(tok_i32[:], [[0, 1]], channel_multiplier=1)
            tok_f = constp.tile([128, 1], F32)
            nc.vector.tensor_copy(tok_f[:], tok_i32[:])
            # cbase[0, e] = e * C
            cb_i32 = constp.tile([1, E], I32)
            nc.gpsimd.iota(cb_i32[:], [[C, E]], channel_multiplier=0)
            cb_f = constp.tile([1, E], F32)
            nc.vector.tensor_copy(cb_f[:], cb_i32[:])
            # wc init payload rows (w=0, idx=NTOK) and bf16 zero tile
            WCA = ZROWS // 128
            wcinit = constp.tile([128, WCA, 2], F32)
            nc.gpsimd.memset(wcinit[:], 0.0)
            nc.gpsimd.memset(wcinit[:, :, 1:2], float(NTOK))
            zt0 = constp.tile([128, H], BF16)
            nc.gpsimd.memset(zt0[:], 0.0)
            # zero-init zc buffers once (pad rows must be finite)
            for l in range(L):
                for a in range(ZROWS // 128):
                    nc.sync.dma_start(zc_d[l].ap()[a * 128:(a + 1) * 128, :],
                                      zt0[:])

            for l in range(L):
                x_src = x_d.ap() if l == 0 else x1_d.ap()
                dst = y_d.ap() if l == L - 1 else x1_d.ap()
                zc = zc_d[l]
                wc = wc_d[l]

                # ---- per-layer constants ----
                wr_sb = lconp.tile([128, KT, E], F32, tag="wr")
                nc.sync.dma_start(wr_sb[:], wr_d.ap()[l].rearrange(
                    "(k p) e -> p k e", p=128))
                br_bc = lconp.tile([128, E], F32, tag="br")
                nc.sync.dma_start(br_bc[:], br_d.ap()[l].unsqueeze(0)
                                  .broadcast_to((128, E)))
                if l == 0:
                    u_sb = lconp.tile([128, KT, NU], F32, tag="uc", bufs=1)
                    nc.sync.dma_start(u_sb[:], uc_d.ap().rearrange(
                        "(k p) u -> p k u", p=128))
                if l == 1:
                    rc_bc = lconp.tile([128, 8 * E], F32, tag="rc", bufs=1)
                    nc.sync.dma_start(rc_bc[:], rc_d.ap().rearrange(
                        "a b -> (a b)").unsqueeze(0).broadcast_to((128, 8 * E)))

                # init wc payload buffer: w=0, idx=NTOK (trash row)
                nc.sync.dma_start(
                    wc.ap().rearrange("(a p) b -> p a b", p=128), wcinit[:])

                # ---- Phase A ----
                w_tiles = []
                tiles_a = [None] * NI
                rsigs = [None] * NI
                obase = cmpp.tile([1, E], F32, tag="ob", bufs=NI + 2)
                nc.vector.tensor_copy(obase[:], cb_f[:])
                if l == 0:
                    zu_tiles, mu_c, sd_c, w0_tiles = [], [], [], []
                for ii in range(NI + 1):
                    if ii < NI:
                        i = ii
                        tsl = slice(i * 128, (i + 1) * 128)
                        xt = xp.tile([128, H], F32, tag="x")
                        nc.sync.dma_start(xt[:], x_src[tsl, :])
                        # residual init: dst rows = x rows
                        nc.sync.dma_start(dst[tsl, :], xt[:])

                        s1 = smp.tile([128, 1], F32, tag="s")
                        nc.vector.tensor_reduce(s1[:], xt[:], AX.X, ALU.add)
                        mu = smp.tile([128, 1], F32, tag="muc", bufs=2 * NI)
                        nc.vector.tensor_scalar_mul(mu[:], s1[:], 1.0 / H)

                        SQC = min(HO, H)
                        nsq = H // SQC
                        s2p = smp.tile([128, max(nsq, 2)], F32, tag="sp")
                        for c in range(nsq):
                            sqps = psp.tile([128, SQC], F32, tag="ps",
                                            name="sqps")
                            nc.scalar.activation(sqps[:],
                                                 xt[:, c * SQC:(c + 1) * SQC],
                                                 AF.Square,
                                                 accum_out=s2p[:, c:c + 1])
                        s2 = smp.tile([128, 1], F32, tag="s")
                        nc.vector.tensor_reduce(s2[:], s2p[:, :nsq], AX.X,
                                                ALU.add)
                        ex2 = smp.tile([128, 1], F32, tag="s")
                        nc.vector.tensor_scalar_mul(ex2[:], s2[:], 1.0 / H)
                        musq = smp.tile([128, 1], F32, tag="s")
                        nc.vector.tensor_mul(musq[:], mu[:], mu[:])
                        var = smp.tile([128, 1], F32, tag="s")
                        nc.vector.tensor_sub(var[:], ex2[:], musq[:])
                        sd = smp.tile([128, 1], F32, tag="sdc", bufs=2 * NI)
                        nc.scalar.activation(sd[:], var[:], AF.Sqrt,
                                             bias=eps_t[:])
                        rsig = smp.tile([128, 1], F32, tag="s")
                        nc.vector.reciprocal(rsig[:], sd[:])
                        rsigs[i] = rsig
                        if l == 0:
                            mu_c.append(mu)
                            sd_c.append(sd)

                        # z in f32 (router/transpose) and bf16 (expert path)
                        z32 = z32p.tile([128, H], F32, tag="z32")
                        nc.vector.tensor_scalar(z32[:], xt[:], mu[:], rsig[:],
                                                ALU.subtract, ALU.mult)
                        zb = zbp.tile([128, H], BF16, tag="zb")
                        nc.vector.tensor_copy(zb[:], z32[:])
                        tiles_a[i] = (z32, zb)

                    if ii > 0:
                        i = ii - 1
                        z32, zb = tiles_a[i]
                        # transpose z32 into zf (f32, router input)
                        zf = zfp.tile([128, KT, 128], F32, tag="zf", name="zf")
                        for kb in range(KT // TB):
                            pt = psp.tile([128, TB, 128], F32, tag="ps",
                                          name="pt")
                            for j in range(TB):
                                k = kb * TB + j
                                nc.tensor.transpose(
                                    pt[:, j, :],
                                    z32[:, k * 128:(k + 1) * 128], ident[:])
                            nc.vector.tensor_copy(
                                zf[:, kb * TB:(kb + 1) * TB, :], pt[:])

                        if l == 0:
                            # router logits: full fp32 matmul (exact)
                            lp = psp.tile([128, E], F32, tag="ps")
                            for k in range(KT):
                                nc.tensor.matmul(lp[:], zf[:, k, :],
                                                 wr_sb[:, k, :],
                                                 start=(k == 0),
                                                 stop=(k == KT - 1))
                            ls = wp.tile([128, E], F32, tag="w")
                            nc.vector.tensor_add(ls[:], lp[:], br_bc[:])
                            # layer-2 surrogate projections
                            pu = psp.tile([128, NU], F32, tag="ps")
                            for k in range(KT):
                                nc.tensor.matmul(pu[:], zf[:, k, :],
                                                 u_sb[:, k, :],
                                                 start=(k == 0),
                                                 stop=(k == KT - 1))
                            zu = wp.tile([128, NU], F32, tag="zu", bufs=2 * NI)
                            nc.vector.tensor_copy(zu[:], pu[:])
                            zu_tiles.append(zu)
                        if l == 1:
                            # exact layer-2 logits from layer-1 projections
                            zu = zu_tiles[i]
                            w0 = w0_tiles[i]
                            mu0 = mu_c[i]
                            sd0 = sd_c[i]
                            t1 = wp.tile([128, E], F32, tag="w")
                            nc.vector.tensor_sub(t1[:], zu[:, 0:4],
                                                 rc_bc[:, 0:4])
                            t2 = wp.tile([128, E], F32, tag="w")
                            nc.vector.tensor_scalar_mul(t2[:], t1[:], sd0[:])
                            t3 = wp.tile([128, E], F32, tag="w")
                            nc.vector.tensor_scalar_mul(t3[:], rc_bc[:, 4:8],
                                                        mu0[:])
                            xA = wp.tile([128, E], F32, tag="w")
                            nc.vector.tensor_add(xA[:], t2[:], t3[:])
                            u16 = wp.tile([128, 4 * E], F32, tag="w16", bufs=4)
                            nc.vector.tensor_add(u16[:], zu[:, 4:4 + 4 * E],
                                                 rc_bc[:, 8:8 + 4 * E])
                            macc = None
                            for e in range(E):
                                te = wp.tile([128, E], F32, tag="w", name="te")
                                nc.vector.tensor_scalar_mul(
                                    te[:], u16[:, 4 * e:4 * e + 4],
                                    w0[:, e:e + 1])
                                if macc is None:
                                    macc = te
                                else:
                                    macc2 = wp.tile([128, E], F32, tag="w",
                                                    name="macc2")
                                    nc.vector.tensor_add(macc2[:], macc[:],
                                                         te[:])
                                    macc = macc2
                            x1A = wp.tile([128, E], F32, tag="w")
                            nc.vector.tensor_add(x1A[:], xA[:], macc[:])
                            m4 = wp.tile([128, E], F32, tag="w")
                            nc.vector.tensor_add(m4[:],
                                                 zu[:, 4 + 4 * E:4 + 5 * E],
                                                 rc_bc[:, 24:28])
                            m4w = wp.tile([128, E], F32, tag="w")
                            nc.vector.tensor_mul(m4w[:], m4[:], w0[:])
                            ms = smp.tile([128, 1], F32, tag="s")
                            nc.vector.tensor_reduce(ms[:], m4w[:], AX.X,
                                                    ALU.add)
                            mux1 = smp.tile([128, 1], F32, tag="s")
                            nc.vector.tensor_add(mux1[:], mu0[:], ms[:])
                            s4 = wp.tile([128, E], F32, tag="w")
                            nc.vector.tensor_scalar_mul(s4[:], rc_bc[:, 4:8],
                                                        mux1[:])
                            l0 = wp.tile([128, E], F32, tag="w")
                            nc.vector.tensor_sub(l0[:], x1A[:], s4[:])
                            l1 = wp.tile([128, E], F32, tag="w")
                            nc.vector.tensor_scalar_mul(l1[:], l0[:],
                                                        rsigs[i][:])
                            ls = wp.tile([128, E], F32, tag="w")
                            nc.vector.tensor_add(ls[:], l1[:], rc_bc[:, 28:32])

                        # top-2 renormalized softmax
                        m1 = smp.tile([128, 1], F32, tag="s")
                        nc.vector.tensor_reduce(m1[:], ls[:], AX.X, ALU.max)
                        nm1 = smp.tile([128, 1], F32, tag="s")
                        nc.vector.tensor_scalar_mul(nm1[:], m1[:], -1.0)
                        selmax = wp.tile([128, E], F32, tag="w")
                        nc.vector.tensor_scalar(selmax[:], ls[:], m1[:], 1e30,
                                                ALU.is_ge, ALU.mult)
                        lmsk = wp.tile([128, E], F32, tag="w")
                        nc.vector.tensor_sub(lmsk[:], ls[:], selmax[:])
                        m2 = smp.tile([128, 1], F32, tag="s")
                        nc.vector.tensor_reduce(m2[:], lmsk[:], AX.X, ALU.max)
                        sel2 = wp.tile([128, E], F32, tag="w")
                        nc.vector.tensor_scalar(sel2[:], ls[:], m2[:], None,
                                                ALU.is_ge)
                        et = wp.tile([128, E], F32, tag="w")
                        nc.scalar.activation(et[:], ls[:], AF.Exp, bias=nm1[:])
                        ew = wp.tile([128, E], F32, tag="w")
                        nc.vector.tensor_mul(ew[:], et[:], sel2[:])
                        ssum = smp.tile([128, 1], F32, tag="s")
                        nc.vector.tensor_reduce(ssum[:], ew[:], AX.X, ALU.add)
                        rs = smp.tile([128, 1], F32, tag="s")
                        nc.vector.reciprocal(rs[:], ssum[:])
                        w_t = wp.tile([128, E], F32, tag="wt", bufs=2 * NI + 2,
                                      name="w_t")
                        nc.vector.tensor_scalar_mul(w_t[:], ew[:], rs[:])
                        w_tiles.append(w_t)
                        if l == 0:
                            w0_tiles.append(w_t)

                        # ---- incremental compaction + slot scatters ----
                        sel_i = cmpp.tile([128, E], F32, tag="sel", bufs=3)
                        nc.vector.tensor_scalar(sel_i[:], w_t[:], 0.0, None,
                                                ALU.is_gt)
                        # dest = (exclusive prefix within tile) + obase, via
                        # two PSUM-accumulated matmuls (integer-exact in f32)
                        pD = psp.tile([128, E], F32, tag="ps", name="pD")
                        nc.tensor.matmul(pD[:], ut128[:], sel_i[:],
                                         start=True, stop=False)
                        nc.tensor.matmul(pD[:], ones_row[:], obase[:],
                                         start=False, stop=True)
                        Dt = cmpp.tile([128, E], F32, tag="Dt", bufs=3)
                        nc.vector.tensor_copy(Dt[:], pD[:])
                        # per-expert totals -> next obase
                        pT = psp.tile([1, E], F32, tag="ps", name="pT")
                        nc.tensor.matmul(pT[:], ones_col[:], sel_i[:],
                                         start=True, stop=True)
                        obase2 = cmpp.tile([1, E], F32, tag="ob",
                                           bufs=NI + 2, name="obase2")
                        nc.vector.tensor_add(obase2[:], obase[:], pT[:])
                        obase = obase2

                        cum = cmpp.tile([128, E], F32, tag="cum", bufs=3)
                        nc.gpsimd.memset(cum[:, 0:1], 0.0)
                        nc.vector.tensor_copy(cum[:, 1:2], sel_i[:, 0:1])
                        nc.vector.tensor_add(cum[:, 2:3], cum[:, 1:2],
                                             sel_i[:, 1:2])
                        nc.vector.tensor_add(cum[:, 3:4], cum[:, 2:3],
                                             sel_i[:, 2:3])
                        tokn = cmpp.tile([128, 1], F32, tag="tokn", bufs=3)
                        nc.vector.tensor_scalar_add(tokn[:], tok_f[:],
                                                    float(128 * i))
                        for s in range(2):
                            eqs = cmpp.tile([128, E], F32, tag="eqs")
                            nc.vector.tensor_scalar(eqs[:], cum[:], float(s),
                                                    None, ALU.is_equal)
                            msk = cmpp.tile([128, E], F32, tag="msk")
                            nc.vector.tensor_mul(msk[:], eqs[:], sel_i[:])
                            dtm = cmpp.tile([128, E], F32, tag="dtm")
                            nc.vector.tensor_mul(dtm[:], msk[:], Dt[:])
                            dsl = cmpp.tile([128, 1], F32, tag="dsl")
                            nc.vector.tensor_reduce(dsl[:], dtm[:], AX.X,
                                                    ALU.add)
                            d_i32 = cmpp.tile([128, 1], I32, tag="di")
                            nc.vector.tensor_copy(d_i32[:], dsl[:])
                            wtm = cmpp.tile([128, E], F32, tag="wtm")
                            nc.vector.tensor_mul(wtm[:], msk[:], w_t[:])
                            pay = cmpp.tile([128, 2], F32, tag="pay")
                            nc.vector.tensor_reduce(pay[:, 0:1], wtm[:], AX.X,
                                                    ALU.add)
                            nc.vector.tensor_copy(pay[:, 1:2], tokn[:])
                            nc.gpsimd.indirect_dma_start(
                                out=zc.ap()[:, :],
                                out_offset=bass.IndirectOffsetOnAxis(
                                    ap=d_i32[:, :1], axis=0),
                                in_=zb[:],
                                in_offset=None,
                            )
                            nc.gpsimd.indirect_dma_start(
                                out=wc.ap()[:, :],
                                out_offset=bass.IndirectOffsetOnAxis(
                                    ap=d_i32[:, :1], axis=0),
                                in_=pay[:],
                                in_offset=None,
                            )

                # ---- Phase B ----
                dst6 = dst.rearrange("a (n h) -> (a n) h", h=HO)
                for e in range(E):
                    zcTs = []
                    wcrs = []
                    for ct in range(NCT):
                        r0 = e * C + ct * 128
                        zcb = zcbp.tile([128, H], BF16, tag="zcb")
                        nc.scalar.dma_start(zcb[:], zc.ap()[r0:r0 + 128, :])
                        wcr = lconp.tile([128, 2], F32, tag="wcr",
                                         bufs=2 * NCT)
                        nc.sync.dma_start(wcr[:], wc.ap()[r0:r0 + 128, :])
                        zcT = ztp.tile([128, KT, 128], BF16, tag="zcT")
                        for kb in range(KT // TB):
                            pt2 = psp.tile([128, TB, 128], BF16, tag="ps",
                                           name="pt2")
                            for j in range(TB):
                                k = kb * TB + j
                                nc.tensor.transpose(
                                    pt2[:, j, :],
                                    zcb[:, k * 128:(k + 1) * 128], identb[:])
                            nc.vector.tensor_copy(
                                zcT[:, kb * TB:(kb + 1) * TB, :], pt2[:])
                        zcTs.append(zcT)
                        wcrs.append(wcr)
                    wmat = we_d.ap()[l, e].rearrange("(k p) n -> p k n", p=128)
                    for ho in range(NHO):
                        osl = slice(ho * HO, (ho + 1) * HO)
                        wchs = []
                        for hf in range(2):
                            wch = wchp.tile([128, KH, HO], BF16, tag="wch")
                            nc.scalar.dma_start(
                                wch[:], wmat[:, hf * KH:(hf + 1) * KH, osl])
                            wchs.append(wch)
                        be_bc = lconp.tile([128, HO], F32, tag="bebc",
                                           bufs=4)
                        nc.sync.dma_start(
                            be_bc[:], be_d.ap()[l, e, osl].unsqueeze(0)
                            .broadcast_to((128, HO)))
                        for ct in range(NCT):
                            pbs = psp.tile([128, HO], F32, tag="ps",
                                           name="pbs")
                            for k in range(KT):
                                nc.tensor.matmul(pbs[:], zcTs[ct][:, k, :],
                                                 wchs[k // KH][:, k % KH, :],
                                                 start=(k == 0),
                                                 stop=(k == KT - 1))
                            wcr = wcrs[ct]
                            tm = tmp_p.tile([128, HO], F32, tag="tm")
                            nc.scalar.activation(tm[:], pbs[:], AF.Copy,
                                                 scale=wcr[:, 0:1])
                            # + w * be (be broadcast row)
                            bet = tmp_p.tile([128, HO], F32, tag="bet",
                                             bufs=2)
                            nc.vector.tensor_scalar(bet[:], be_bc[:],
                                                    wcr[:, 0:1], None,
                                                    ALU.mult)
                            nc.vector.tensor_add(tm[:], tm[:], bet[:])
                            d6 = cmpp.tile([128, 1], F32, tag="d6")
                            nc.vector.tensor_scalar(d6[:], wcr[:, 1:2],
                                                    float(NHO), float(ho),
                                                    ALU.mult, ALU.add)
                            d6i = cmpp.tile([128, 1], I32, tag="d6i")
                            nc.vector.tensor_copy(d6i[:], d6[:])
                            nc.gpsimd.indirect_dma_start(
                                out=dst6[:, :],
                                out_offset=bass.IndirectOffsetOnAxis(
                                    ap=d6i[:, :1], axis=0),
                                in_=tm[:],
                                in_offset=None,
                                compute_op=ALU.add,
                            )

    nc.compile()
    return nc


# ======== kernel entry points ========

import ml_dtypes

N_CORES = 8
B, T, H, E, L = 4, 2048, 3072, 4, 2
NTOK_TOTAL = B * T
NTOK = NTOK_TOTAL // N_CORES
CAP = 640

_nc_cache = {}


def _get_nc(ntok=NTOK, cap=CAP):
    key = (ntok, cap)
    if key not in _nc_cache:
        _nc_cache[key] = build_moe_routed_kernel(ntok, H, E, L, 512, cap)
    return _nc_cache[key]


def _fold_consts(ln_g, ln_b, Wr, br, We, be):
    """Fold ln_g/ln_b into the weights (fp64), plus layer-2 router surrogate
    constants. Device z is raw (x-mu)*rsig."""
    g = ln_g.astype(np.float64)
    b = ln_b.astype(np.float64)
    Wr64 = Wr.astype(np.float64)
    We64 = We.astype(np.float64)
    be64 = be.astype(np.float64)
    Wr_f = Wr64 * g[:, :, None]                       # [L, H, E]
    br_f = br.astype(np.float64) + np.einsum("lh,lhe->le", b, Wr64)
    We_f = We64 * g[:, None, :, None]                 # [L, E, H, H]
    be_f = be64 + np.einsum("lh,lehd->led", b, We64)
    # surrogate: logits2 = rsig2*(x1@A - mu(x1)*sumA) + br_f[1]
    A = Wr_f[1]                                       # [H, E]
    cols = [A]
    for e in range(E):
        cols.append(We_f[0, e] @ A)                   # [H, E]
    for e in range(E):
        cols.append(We_f[0, e].mean(axis=1)[:, None])  # [H, 1]
    Ucomb = np.concatenate(cols, axis=1).astype(np.float32)
    rconst = np.zeros((8, E), np.float64)
    rconst[1] = A.sum(0)
    for e in range(E):
        rconst[2 + e] = be_f[0, e] @ A
    rconst[6] = [be_f[0, e].mean() for e in range(E)]
    rconst[7] = br_f[1]
    return (
        Wr_f.astype(np.float32),
        br_f.astype(np.float32),
        We_f.astype(np.float32).astype(ml_dtypes.bfloat16),
        be_f.astype(np.float32),
        Ucomb,
        rconst.astype(np.float32),
    )


def _make_in_maps(x, ln_g, ln_b, Wr, br, We, be, n_cores=N_CORES):
    ntok = NTOK_TOTAL // n_cores
    xf = np.ascontiguousarray(x.reshape(NTOK_TOTAL, H), dtype=np.float32)
    Wr_f, br_f, We_f, be_f, Ucomb, rconst = _fold_consts(
        ln_g, ln_b, Wr, br, We, be)
    shared = {
        "Wr": Wr_f, "br": br_f, "We": We_f, "be": be_f,
        "Ucomb": Ucomb, "rconst": rconst,
    }
    return [
        {"x": xf[c * ntok:(c + 1) * ntok], **shared}
        for c in range(n_cores)
    ]


def kernel(x, ln_g, ln_b, Wr, br, We, be):
    from concourse.bass_utils import run_bass_kernel_spmd
    nc = _get_nc()
    in_maps = _make_in_maps(x, ln_g, ln_b, Wr, br, We, be)
    res = run_bass_kernel_spmd(nc, in_maps, core_ids=list(range(N_CORES)))
    y = np.concatenate([res.results[c]["y"][:NTOK] for c in range(N_CORES)],
                       axis=0)
    return y.reshape(B, T, H).astype(np.float32)


def run_profiled(inputs):
    from concourse.bass_utils import run_bass_kernel_spmd
    nc = _get_nc()
    in_maps = _make_in_maps(**inputs)
    return run_bass_kernel_spmd(nc, in_maps, core_ids=list(range(N_CORES)),
                                trace=True)
